# revision 13
# baseline (speedup 1.0000x reference)
"""NT-Xent contrastive loss on 8 Trainium2 NeuronCores (Bass/Tile), v2.

Same slab-cover strategy as v1 (no collectives): core c loads slabs
S_c = {c, c+1, c+2, c+4} (mod 8); every slab pair meets on some core, so
each of the 36 unique 1024x1024 sim blocks is computed once globally
(cores 0-3 dedup block B4 on host). v2 rebuilds the per-core kernel for
speed:

  * fp8(e4m3) inputs: embT pre-transposed AND pre-laid-out on host into
    the exact SBUF tile layout [128, slab, kchunk, row] so each slab DMA
    is 128 x 16 KiB contiguous descriptors (8 MiB/core vs 32 MiB in v1).
  * All heavy matmuls run fp8 DoubleRow (K=256 per instruction): head
    projection, sim blocks, and exp-column-sums (over mb-pair tiles,
    ones[128,2,1] x exp[128,2,512]).
  * L2-normalize: normsq via ones-matmul on bf16 squares; rsqrt done as
    Exp(-0.5*Ln(x)) on ScalarE (Rsqrt activation is banned; Ln+Exp share
    one ACT table set so there is no table thrashing); 1/norm broadcast
    via K=1 ones-row matmul; normalize multiply emits fp8 t_on directly.
  * exp tiles stored fp8e5 (max 57344 > e^10.5) in mb-PAIRED tiles
    [128, 2, 1024] so column sums can use DoubleRow.
  * Diagonal of the diag block: sim_ii as computed by the fp8 matmul is
    exactly sum_d u8[d,i]^2, so we recompute it cheaply (DVE square +
    ones-matmul + ACT exp) and subtract on host - no mask extraction.
  * pos term: elementwise product of fp8 slabs 0,3 + ones-matmul;
    log(pos) = 10*possim exactly (no exp).
  * PSUM budget = 8 banks exactly: head 2 (dh-sequential chains), sim 4
    (double-buffered [128,1024] + broadcast tiles share the pool),
    normsq 1, colsum 1 (nb-sequential chains over persistent exp pairs).
"""
import numpy as np
import ml_dtypes

SLOTS = [(c, (c + 1) % 8, (c + 2) % 8, (c + 4) % 8) for c in range(8)]
WSCALE = 32.0  # power of two; normalize() cancels it exactly

_CACHE = {}


def _build():
    if "nc" in _CACHE:
        return _CACHE["nc"]
    import concourse.bacc as bacc
    import concourse.tile as tile
    import concourse.mybir as mybir

    F32 = mybir.dt.float32
    BF16 = mybir.dt.bfloat16
    F8E4 = mybir.dt.float8e4
    F8E5 = mybir.dt.float8e5
    AF = mybir.ActivationFunctionType
    ALU = mybir.AluOpType
    DR = mybir.MatmulPerfMode.DoubleRow

    # Steer walrus act-table selection: keep Exp/Ln only in the combined
    # natural_log_exp_and_others set so the kernel needs ONE table load
    # instead of thrashing exp_and_others <-> natural_log (1.28us each).
    _orig_gat = bacc.get_activation_tables

    def _gat(arch):
        t = _orig_gat(arch)
        for name, fns in t.items():
            if name != "natural_log_exp_and_others":
                fns.discard(mybir.ActivationFunctionType.Exp)
                fns.discard(mybir.ActivationFunctionType.Ln)
        return t

    bacc.get_activation_tables = _gat

    nc = bacc.Bacc("TRN2", num_devices=8, debug=False)
    a_emb = nc.dram_tensor("emb8", [128, 4, 16, 1024], F8E4,
                           kind="ExternalInput").ap()
    a_W = nc.dram_tensor("W8", [128, 16, 256], F8E4, kind="ExternalInput").ap()
    a_b = nc.dram_tensor("bS", [1, 256], BF16, kind="ExternalInput").ap()
    a_oc = nc.dram_tensor("ones_col", [128, 1], BF16, kind="ExternalInput").ap()
    a_or = nc.dram_tensor("ones_row", [1, 512], BF16, kind="ExternalInput").ap()
    a_o8 = nc.dram_tensor("ones8", [128, 32], F8E5, kind="ExternalInput").ap()
    o_rp = nc.dram_tensor("rowpart", [128, 40], F32, kind="ExternalOutput").ap()
    o_cp = nc.dram_tensor("colpart", [4, 1024], F32, kind="ExternalOutput").ap()
    o_dg = nc.dram_tensor("diagexp", [1, 1024], F32, kind="ExternalOutput").ap()
    o_ps = nc.dram_tensor("possim", [1, 1024], F32, kind="ExternalOutput").ap()

    with tile.TileContext(nc) as tc:
        with tc.tile_pool(name="sb", bufs=1) as sb, \
             tc.tile_pool(name="emb", bufs=4) as embp, \
             tc.tile_pool(name="hp", bufs=2) as hp, \
             tc.tile_pool(name="sq", bufs=2) as sqp, \
             tc.tile_pool(name="rn", bufs=2) as rnp, \
             tc.tile_pool(name="ln", bufs=2) as lnp, \
             tc.tile_pool(name="expp", bufs=9) as expp, \
             tc.tile_pool(name="headp", bufs=2, space="PSUM") as headp, \
             tc.tile_pool(name="simp", bufs=2, space="PSUM") as simp, \
             tc.tile_pool(name="nsp", bufs=1, space="PSUM") as nsp, \
             tc.tile_pool(name="csp", bufs=1, space="PSUM") as csp:

            t_W = sb.tile([128, 16, 256], F8E4, name="t_W")
            nc.scalar.dma_start(t_W[:], a_W[:])
            t_b = sb.tile([1, 256], BF16, name="t_b")
            nc.gpsimd.dma_start(t_b[:], a_b[:])
            t_e = []
            te0 = embp.tile([128, 16, 1024], F8E4, name="t_e0", tag="emb")
            for ch, eng in zip(range(4), (nc.gpsimd, nc.sync, nc.sync,
                                          nc.scalar)):
                eng.dma_start(te0[:, 4 * ch:4 * ch + 4, :],
                              a_emb[:, 0, 4 * ch:4 * ch + 4, :])
            t_e.append(te0)
            t_oc = sb.tile([128, 1], BF16, name="t_oc")
            nc.gpsimd.dma_start(t_oc[:], a_oc[:])
            t_or = sb.tile([1, 512], BF16, name="t_or")
            nc.gpsimd.dma_start(t_or[:], a_or[:])
            t_o8 = sb.tile([128, 2, 16], F8E5, name="t_o8")
            nc.gpsimd.dma_start(t_o8[:], a_o8.rearrange("p (a o) -> p a o", o=16))

            # prefetch remaining emb slabs
            for k in range(1, 4):
                te = embp.tile([128, 16, 1024], F8E4, name=f"t_e{k}", tag="emb")
                nc.sync.dma_start(te[:], a_emb[:, k, :, :])
                t_e.append(te)

            # persistent normalized slabs (fp8) and staging accumulators
            t_on = [sb.tile([128, 2, 1024], F8E4, name=f"t_on{k}")
                    for k in range(4)]
            rp_st = sb.tile([128, 5, 8], F32, name="rp_st")
            cp_st = sb.tile([1, 4096], F32, name="cp_st")
            dg_st = sb.tile([1, 1024], F32, name="dg_st")
            ps_st = sb.tile([1, 1024], F32, name="ps_st")

            def head_chain(k, th, tsq, h, dh):
                """one (h, dh) quarter of slab k's head projection."""
                ph = headp.tile([128, 512], F32, name="p_h", tag="head")
                for j in range(8):
                    nc.tensor.matmul(
                        ph[:],
                        t_W[:, 2 * j:2 * j + 2, dh * 128:(dh + 1) * 128],
                        t_e[k][:, 2 * j:2 * j + 2, h * 512:(h + 1) * 512],
                        start=(j == 0), stop=False, perf_mode=DR)
                # bias: + b[d] * ones_row  (K=1 bf16 matmul)
                nc.tensor.matmul(
                    ph[:], t_b[0:1, dh * 128:(dh + 1) * 128],
                    t_or[0:1, :], start=False, stop=True)
                nc.vector.tensor_copy(
                    th[:, dh, h * 512:(h + 1) * 512], ph[:])
                nc.vector.tensor_tensor(
                    tsq[:, dh, h * 512:(h + 1) * 512],
                    th[:, dh, h * 512:(h + 1) * 512],
                    th[:, dh, h * 512:(h + 1) * 512], ALU.mult)

            def norm_half(tsq, rn, h):
                """normsq + rsqrt (Ln,Exp) for rows h*512..h*512+511."""
                pns = nsp.tile([1, 512], F32, name="p_ns", tag="ns")
                for dh in range(2):
                    nc.tensor.matmul(
                        pns[:], t_oc[:], tsq[:, dh, h * 512:(h + 1) * 512],
                        start=(dh == 0), stop=(dh == 1))
                tln = lnp.tile([1, 512], F32, name="t_ln", tag="ln")
                nc.scalar.activation(tln[:], pns[:], AF.Ln)
                nc.scalar.activation(rn[0:1, h * 512:(h + 1) * 512],
                                     tln[:], AF.Exp, scale=-0.5)

            def stage_finish(k, th, rn):
                """broadcast 1/norm and emit the fp8 normalized slab."""
                for h in range(2):
                    pbc = headp.tile([128, 512], F32, name="p_bc", tag="head")
                    nc.tensor.matmul(pbc[:], t_or[0:1, 0:128],
                                     rn[0:1, h * 512:(h + 1) * 512],
                                     start=True, stop=True)
                    for dh in range(2):
                        nc.vector.tensor_tensor(
                            t_on[k][:, dh, h * 512:(h + 1) * 512],
                            th[:, dh, h * 512:(h + 1) * 512],
                            pbc[:], ALU.mult)

            def sim_pair(bslot, a, bm, pair):
                """two mb tiles of a sim block -> one fp8e5 exp pair tile."""
                texp = expp.tile([128, 2, 1024], F8E5, name="t_exp", tag="exp")
                for half in range(2):
                    mb = 2 * pair + half
                    psim = simp.tile([128, 1024], F32, name="p_sim", tag="sim")
                    for nb in range(2):
                        nc.tensor.matmul(
                            psim[:, nb * 512:(nb + 1) * 512],
                            t_on[a][:, :, mb * 128:(mb + 1) * 128],
                            t_on[bm][:, :, nb * 512:(nb + 1) * 512],
                            start=True, stop=True, perf_mode=DR)
                    nc.scalar.activation(
                        texp[:, half, :], psim[:], AF.Exp, scale=10.0,
                        accum_out=rp_st[:, bslot, mb:mb + 1])
                return texp

            def block_cs(bslot, texps):
                """column sums of a block's exp pair tiles."""
                for nb in range(2):
                    pcs = csp.tile([1, 512], F32, name="p_cs", tag="cs")
                    for pair in range(4):
                        nc.tensor.matmul(
                            pcs[:], t_o8[:, :, 0:1],
                            texps[pair][:, :, nb * 512:(nb + 1) * 512],
                            start=(pair == 0), stop=(pair == 3),
                            perf_mode=DR)
                    nc.vector.tensor_copy(
                        cp_st[0:1, 1024 * (bslot - 1) + nb * 512:
                              1024 * (bslot - 1) + (nb + 1) * 512],
                        pcs[:])

            def phase(bslot, a, bm, nxt=None, cs_prev=None):
                """block (bslot): 4 sim pairs, each followed by one head
                chain of the NEXT slab's stage, so the strict-FIFO PE queue
                always has head work while ACT drains the exp backlog and
                frees sim-psum buffers. Stage k+1's rsqrt chain is emitted
                mid-phase so its ACT ops sit ahead of half the exps."""
                th = tsq = rn = None
                if nxt is not None:
                    th = hp.tile([128, 2, 1024], BF16, name="t_h", tag="th")
                    tsq = sqp.tile([128, 2, 1024], BF16, name="t_sq", tag="sq")
                    rn = rnp.tile([1, 1024], BF16, name="t_rn", tag="rn")
                texps = []
                for pair in range(4):
                    texps.append(sim_pair(bslot, a, bm, pair))
                    if nxt is not None:
                        head_chain(nxt, th, tsq, h=pair // 2, dh=pair % 2)
                        if pair == 1:
                            norm_half(tsq, rn, 0)
                        elif pair == 3:
                            norm_half(tsq, rn, 1)
                if nxt is not None:
                    stage_finish(nxt, th, rn)
                if cs_prev is not None:
                    block_cs(*cs_prev)
                return texps

            def colreduce_exp(src8, dst, scale):
                """dst[1,1024] = f(sum_d src8a[d,:]*src8b[d,:])."""
                tq = sqp.tile([128, 2, 1024], BF16, name="t_q", tag="sq")
                nc.vector.tensor_tensor(tq[:], src8[0][:], src8[1][:],
                                        ALU.mult)
                for nb in range(2):
                    pr = nsp.tile([1, 512], F32, name="p_r", tag="ns")
                    for dh in range(2):
                        nc.tensor.matmul(
                            pr[:], t_oc[:], tq[:, dh, nb * 512:(nb + 1) * 512],
                            start=(dh == 0), stop=(dh == 1))
                    if scale is None:
                        nc.vector.tensor_copy(
                            dst[0:1, nb * 512:(nb + 1) * 512], pr[:])
                    else:
                        nc.scalar.activation(
                            dst[0:1, nb * 512:(nb + 1) * 512], pr[:],
                            AF.Exp, scale=scale)

            # slab 0 head alone, then software-pipelined phases
            th0 = hp.tile([128, 2, 1024], BF16, name="t_h", tag="th")
            tsq0 = sqp.tile([128, 2, 1024], BF16, name="t_sq", tag="sq")
            rn0 = rnp.tile([1, 1024], BF16, name="t_rn", tag="rn")
            for h in range(2):
                for dh in range(2):
                    head_chain(0, th0, tsq0, h, dh)
                norm_half(tsq0, rn0, h)
            stage_finish(0, th0, rn0)

            tx0 = phase(0, 0, 0, nxt=1)
            # diag exp values: exp(10 * |u8_i|^2) == exp(10 * sim_ii)
            colreduce_exp((t_on[0], t_on[0]), dg_st, 10.0)
            tx1 = phase(1, 0, 1, nxt=2)
            tx2 = phase(2, 0, 2, nxt=3, cs_prev=(1, tx1))
            # pos: possim_i = sum_d u0[d,i]*u3[d,i]; host uses 10*possim
            colreduce_exp((t_on[0], t_on[3]), ps_st, None)
            tx3 = phase(3, 1, 3, cs_prev=(2, tx2))
            tx4 = phase(4, 0, 3, cs_prev=(3, tx3))
            block_cs(4, tx4)

            nc.gpsimd.dma_start(o_rp[:],
                                rp_st[:].rearrange("p a b -> p (a b)"))
            nc.gpsimd.dma_start(o_cp.rearrange("a r -> (a r)")[None, :],
                                cp_st[:])
            nc.gpsimd.dma_start(o_dg[:], dg_st[:])
            nc.gpsimd.dma_start(o_ps[:], ps_st[:])

    try:
        nc.compile()
    finally:
        bacc.get_activation_tables = _orig_gat
    _CACHE["nc"] = nc
    return nc


def _host_inputs(embedded_data, W, b):
    emb = np.asarray(embedded_data, dtype=np.float32)      # [8192, 2048]
    W = np.asarray(W, dtype=np.float32)
    b = np.asarray(b, dtype=np.float32)
    # slab s tile layout: [128(p), 16(kc), 1024(r)], value = emb[r0+r, 128*kc+p]
    embT = np.ascontiguousarray(emb.T)                     # [2048, 8192]
    emb8 = embT.reshape(16, 128, 8192).transpose(1, 0, 2)  # [128, 16, 8192]
    emb8 = emb8.astype(ml_dtypes.float8_e4m3)
    W8 = (W * WSCALE).reshape(16, 128, 256).transpose(1, 0, 2)
    W8 = np.ascontiguousarray(W8).astype(ml_dtypes.float8_e4m3)
    bS = np.ascontiguousarray((b * WSCALE).reshape(1, 256)).astype(
        ml_dtypes.bfloat16)
    ones_col = np.ones((128, 1), ml_dtypes.bfloat16)
    ones_row = np.ones((1, 512), ml_dtypes.bfloat16)
    ones8 = np.ones((128, 32), ml_dtypes.float8_e5m2)
    in_maps = []
    for c in range(8):
        sl = np.stack([emb8[:, :, 1024 * s:1024 * (s + 1)] for s in SLOTS[c]],
                      axis=1)                              # [128, 4, 16, 1024]
        in_maps.append({"emb8": np.ascontiguousarray(sl), "W8": W8, "bS": bS,
                        "ones_col": ones_col, "ones_row": ones_row,
                        "ones8": ones8})
    return in_maps


def _combine(results):
    neg = np.zeros(8192, np.float64)
    pos = np.zeros(8192, np.float64)
    for c in range(8):
        S = SLOTS[c]
        rp = results[c]["rowpart"].astype(np.float64)
        rp = rp.reshape(128, 5, 8).transpose(1, 2, 0).reshape(5, 1024)
        cp = results[c]["colpart"].astype(np.float64)
        dg = results[c]["diagexp"].astype(np.float64).ravel()
        sl = [np.s_[1024 * s:1024 * (s + 1)] for s in S]
        neg[sl[0]] += rp[0] - dg          # diag block, self-sim removed
        neg[sl[0]] += rp[1]; neg[sl[1]] += cp[0]   # B1 (0,1)
        neg[sl[0]] += rp[2]; neg[sl[2]] += cp[1]   # B2 (0,2)
        neg[sl[1]] += rp[3]; neg[sl[3]] += cp[2]   # B3 (1,3)
        if c < 4:                                   # B4 (0,3) dedup: cores 0-3
            neg[sl[0]] += rp[4]; neg[sl[3]] += cp[3]
            ps = results[c]["possim"].astype(np.float64).ravel()
            pos[sl[0]] = ps
            pos[1024 * S[3]:1024 * (S[3] + 1)] = ps
    loss = -np.mean(10.0 * pos - np.log(neg))
    return np.float32(loss)


def run(embedded_data, W, b, trace=False):
    from concourse import bass_utils
    nc = _build()
    in_maps = _host_inputs(embedded_data, W, b)
    res = bass_utils.run_bass_kernel_spmd(nc, in_maps, core_ids=list(range(8)),
                                          trace=trace)
    return _combine(res.results), res


def kernel(embedded_data, W, b):
    loss, _ = run(embedded_data, W, b, trace=False)
    return np.asarray(loss, dtype=np.float32)


# revision 14
# speedup vs baseline: 1.0391x; 1.0391x over previous
"""NT-Xent contrastive loss on 8 Trainium2 NeuronCores (Bass/Tile), v2.

Same slab-cover strategy as v1 (no collectives): core c loads slabs
S_c = {c, c+1, c+2, c+4} (mod 8); every slab pair meets on some core, so
each of the 36 unique 1024x1024 sim blocks is computed once globally
(cores 0-3 dedup block B4 on host). v2 rebuilds the per-core kernel for
speed:

  * fp8(e4m3) inputs: embT pre-transposed AND pre-laid-out on host into
    the exact SBUF tile layout [128, slab, kchunk, row] so each slab DMA
    is 128 x 16 KiB contiguous descriptors (8 MiB/core vs 32 MiB in v1).
  * All heavy matmuls run fp8 DoubleRow (K=256 per instruction): head
    projection, sim blocks, and exp-column-sums (over mb-pair tiles,
    ones[128,2,1] x exp[128,2,512]).
  * L2-normalize: normsq via ones-matmul on bf16 squares; rsqrt done as
    Exp(-0.5*Ln(x)) on ScalarE (Rsqrt activation is banned; Ln+Exp share
    one ACT table set so there is no table thrashing); 1/norm broadcast
    via K=1 ones-row matmul; normalize multiply emits fp8 t_on directly.
  * exp tiles stored fp8e5 (max 57344 > e^10.5) in mb-PAIRED tiles
    [128, 2, 1024] so column sums can use DoubleRow.
  * Diagonal of the diag block: sim_ii as computed by the fp8 matmul is
    exactly sum_d u8[d,i]^2, so we recompute it cheaply (DVE square +
    ones-matmul + ACT exp) and subtract on host - no mask extraction.
  * pos term: elementwise product of fp8 slabs 0,3 + ones-matmul;
    log(pos) = 10*possim exactly (no exp).
  * PSUM budget = 8 banks exactly: head 2 (dh-sequential chains), sim 4
    (double-buffered [128,1024] + broadcast tiles share the pool),
    normsq 1, colsum 1 (nb-sequential chains over persistent exp pairs).
"""
import numpy as np
import ml_dtypes

SLOTS = [(c, (c + 1) % 8, (c + 2) % 8, (c + 4) % 8) for c in range(8)]
WSCALE = 32.0  # power of two; normalize() cancels it exactly

_CACHE = {}


def _build():
    if "nc" in _CACHE:
        return _CACHE["nc"]
    import concourse.bacc as bacc
    import concourse.tile as tile
    import concourse.mybir as mybir

    F32 = mybir.dt.float32
    BF16 = mybir.dt.bfloat16
    F8E4 = mybir.dt.float8e4
    F8E5 = mybir.dt.float8e5
    AF = mybir.ActivationFunctionType
    ALU = mybir.AluOpType
    DR = mybir.MatmulPerfMode.DoubleRow

    # Steer walrus act-table selection: keep Exp/Ln only in the combined
    # natural_log_exp_and_others set so the kernel needs ONE table load
    # instead of thrashing exp_and_others <-> natural_log (1.28us each).
    _orig_gat = bacc.get_activation_tables

    def _gat(arch):
        t = _orig_gat(arch)
        for name, fns in t.items():
            if name != "natural_log_exp_and_others":
                fns.discard(mybir.ActivationFunctionType.Exp)
                fns.discard(mybir.ActivationFunctionType.Ln)
        return t

    bacc.get_activation_tables = _gat

    nc = bacc.Bacc("TRN2", num_devices=8, debug=False)
    a_emb = nc.dram_tensor("emb8", [128, 4, 16, 1024], F8E4,
                           kind="ExternalInput").ap()
    a_W = nc.dram_tensor("W8", [128, 16, 256], F8E4, kind="ExternalInput").ap()
    a_b = nc.dram_tensor("bS", [128, 2], F32, kind="ExternalInput").ap()
    a_oc = nc.dram_tensor("ones_col", [128, 1], BF16, kind="ExternalInput").ap()
    a_or = nc.dram_tensor("ones_row", [1, 512], BF16, kind="ExternalInput").ap()
    a_o8 = nc.dram_tensor("ones8", [128, 32], F8E5, kind="ExternalInput").ap()
    o_rp = nc.dram_tensor("rowpart", [128, 40], F32, kind="ExternalOutput").ap()
    o_cp = nc.dram_tensor("colpart", [4, 1024], F32, kind="ExternalOutput").ap()
    o_dg = nc.dram_tensor("diagexp", [1, 1024], F32, kind="ExternalOutput").ap()
    o_ps = nc.dram_tensor("possim", [1, 1024], F32, kind="ExternalOutput").ap()

    with tile.TileContext(nc) as tc:
        with tc.tile_pool(name="sb", bufs=1) as sb, \
             tc.tile_pool(name="emb", bufs=4) as embp, \
             tc.tile_pool(name="hp", bufs=2) as hp, \
             tc.tile_pool(name="sq", bufs=2) as sqp, \
             tc.tile_pool(name="rn", bufs=2) as rnp, \
             tc.tile_pool(name="ln", bufs=2) as lnp, \
             tc.tile_pool(name="expp", bufs=9) as expp, \
             tc.tile_pool(name="headp", bufs=2, space="PSUM") as headp, \
             tc.tile_pool(name="simp", bufs=2, space="PSUM") as simp, \
             tc.tile_pool(name="nsp", bufs=1, space="PSUM") as nsp, \
             tc.tile_pool(name="csp", bufs=1, space="PSUM") as csp:

            t_W = sb.tile([128, 16, 256], F8E4, name="t_W")
            nc.scalar.dma_start(t_W[:], a_W[:])
            t_b = sb.tile([128, 2], F32, name="t_b")
            nc.gpsimd.dma_start(t_b[:], a_b[:])
            t_e = []
            te0 = embp.tile([128, 16, 1024], F8E4, name="t_e0", tag="emb")
            for ch, eng in zip(range(4), (nc.gpsimd, nc.sync, nc.sync,
                                          nc.scalar)):
                eng.dma_start(te0[:, 4 * ch:4 * ch + 4, :],
                              a_emb[:, 0, 4 * ch:4 * ch + 4, :])
            t_e.append(te0)
            t_oc = sb.tile([128, 1], BF16, name="t_oc")
            nc.gpsimd.dma_start(t_oc[:], a_oc[:])
            t_or = sb.tile([1, 512], BF16, name="t_or")
            nc.gpsimd.dma_start(t_or[:], a_or[:])
            t_o8 = sb.tile([128, 2, 16], F8E5, name="t_o8")
            nc.gpsimd.dma_start(t_o8[:], a_o8.rearrange("p (a o) -> p a o", o=16))

            # prefetch remaining emb slabs
            for k in range(1, 4):
                te = embp.tile([128, 16, 1024], F8E4, name=f"t_e{k}", tag="emb")
                nc.sync.dma_start(te[:], a_emb[:, k, :, :])
                t_e.append(te)

            # persistent normalized slabs (fp8) and staging accumulators
            t_on = [sb.tile([128, 2, 1024], F8E4, name=f"t_on{k}")
                    for k in range(4)]
            rp_st = sb.tile([128, 5, 8], F32, name="rp_st")
            cp_st = sb.tile([1, 4096], F32, name="cp_st")
            dg_st = sb.tile([1, 1024], F32, name="dg_st")
            ps_st = sb.tile([1, 1024], F32, name="ps_st")

            def head_chain(k, th, tsq, h, dh):
                """one (h, dh) quarter of slab k's head projection."""
                ph = headp.tile([128, 512], F32, name="p_h", tag="head")
                for j in range(8):
                    nc.tensor.matmul(
                        ph[:],
                        t_W[:, 2 * j:2 * j + 2, dh * 128:(dh + 1) * 128],
                        t_e[k][:, 2 * j:2 * j + 2, h * 512:(h + 1) * 512],
                        start=(j == 0), stop=(j == 7), perf_mode=DR)
                # evacuate + bias add (per-partition scalar b[d]) in one op
                nc.vector.tensor_scalar_add(
                    th[:, dh, h * 512:(h + 1) * 512], ph[:],
                    t_b[:, dh:dh + 1])
                nc.vector.tensor_tensor(
                    tsq[:, dh, h * 512:(h + 1) * 512],
                    th[:, dh, h * 512:(h + 1) * 512],
                    th[:, dh, h * 512:(h + 1) * 512], ALU.mult)

            def norm_half(tsq, rn, h):
                """normsq + rsqrt (Ln,Exp) for rows h*512..h*512+511."""
                pns = nsp.tile([1, 512], F32, name="p_ns", tag="ns")
                for dh in range(2):
                    nc.tensor.matmul(
                        pns[:], t_oc[:], tsq[:, dh, h * 512:(h + 1) * 512],
                        start=(dh == 0), stop=(dh == 1))
                tln = lnp.tile([1, 512], F32, name="t_ln", tag="ln")
                nc.scalar.activation(tln[:], pns[:], AF.Ln)
                nc.scalar.activation(rn[0:1, h * 512:(h + 1) * 512],
                                     tln[:], AF.Exp, scale=-0.5)

            def stage_finish(k, th, rn, h):
                """broadcast 1/norm, emit fp8 normalized half-slab h."""
                pbc = headp.tile([128, 512], F32, name="p_bc", tag="head")
                nc.tensor.matmul(pbc[:], t_or[0:1, 0:128],
                                 rn[0:1, h * 512:(h + 1) * 512],
                                 start=True, stop=True)
                for dh in range(2):
                    nc.vector.tensor_tensor(
                        t_on[k][:, dh, h * 512:(h + 1) * 512],
                        th[:, dh, h * 512:(h + 1) * 512],
                        pbc[:], ALU.mult)

            def sim_pair(bslot, a, bm, pair):
                """two mb tiles of a sim block -> one fp8e5 exp pair tile."""
                texp = expp.tile([128, 2, 1024], F8E5, name="t_exp", tag="exp")
                for half in range(2):
                    mb = 2 * pair + half
                    psim = simp.tile([128, 1024], F32, name="p_sim", tag="sim")
                    for nb in range(2):
                        nc.tensor.matmul(
                            psim[:, nb * 512:(nb + 1) * 512],
                            t_on[a][:, :, mb * 128:(mb + 1) * 128],
                            t_on[bm][:, :, nb * 512:(nb + 1) * 512],
                            start=True, stop=True, perf_mode=DR)
                    nc.scalar.activation(
                        texp[:, half, :], psim[:], AF.Exp, scale=10.0,
                        accum_out=rp_st[:, bslot, mb:mb + 1])
                return texp

            def block_cs(bslot, texps):
                """column sums of a block's exp pair tiles."""
                for nb in range(2):
                    pcs = csp.tile([1, 512], F32, name="p_cs", tag="cs")
                    for pair in range(4):
                        nc.tensor.matmul(
                            pcs[:], t_o8[:, :, 0:1],
                            texps[pair][:, :, nb * 512:(nb + 1) * 512],
                            start=(pair == 0), stop=(pair == 3),
                            perf_mode=DR)
                    nc.vector.tensor_copy(
                        cp_st[0:1, 1024 * (bslot - 1) + nb * 512:
                              1024 * (bslot - 1) + (nb + 1) * 512],
                        pcs[:])

            def phase(bslot, a, bm, nxt=None, cs_prev=None):
                """block (bslot): 4 sim pairs, each followed by one head
                chain of the NEXT slab's stage, so the strict-FIFO PE queue
                always has head work while ACT drains the exp backlog and
                frees sim-psum buffers. Stage k+1's rsqrt chain is emitted
                mid-phase so its ACT ops sit ahead of half the exps."""
                th = tsq = rn = None
                if cs_prev is not None:
                    block_cs(*cs_prev)
                if nxt is not None:
                    th = hp.tile([128, 2, 1024], BF16, name="t_h", tag="th")
                    tsq = sqp.tile([128, 2, 1024], BF16, name="t_sq", tag="sq")
                    rn = rnp.tile([1, 1024], BF16, name="t_rn", tag="rn")
                texps = []
                for pair in range(4):
                    texps.append(sim_pair(bslot, a, bm, pair))
                    if nxt is not None:
                        head_chain(nxt, th, tsq, h=pair // 2, dh=pair % 2)
                        if pair == 1:
                            norm_half(tsq, rn, 0)
                        elif pair == 2:
                            stage_finish(nxt, th, rn, 0)
                        elif pair == 3:
                            norm_half(tsq, rn, 1)
                if nxt is not None:
                    stage_finish(nxt, th, rn, 1)
                return texps

            def colreduce_exp(src8, dst, scale):
                """dst[1,1024] = f(sum_d src8a[d,:]*src8b[d,:])."""
                tq = sqp.tile([128, 2, 1024], BF16, name="t_q", tag="sq")
                nc.vector.tensor_tensor(tq[:], src8[0][:], src8[1][:],
                                        ALU.mult)
                for nb in range(2):
                    pr = nsp.tile([1, 512], F32, name="p_r", tag="ns")
                    for dh in range(2):
                        nc.tensor.matmul(
                            pr[:], t_oc[:], tq[:, dh, nb * 512:(nb + 1) * 512],
                            start=(dh == 0), stop=(dh == 1))
                    if scale is None:
                        nc.vector.tensor_copy(
                            dst[0:1, nb * 512:(nb + 1) * 512], pr[:])
                    else:
                        nc.scalar.activation(
                            dst[0:1, nb * 512:(nb + 1) * 512], pr[:],
                            AF.Exp, scale=scale)

            # slab 0 head alone, then software-pipelined phases
            th0 = hp.tile([128, 2, 1024], BF16, name="t_h", tag="th")
            tsq0 = sqp.tile([128, 2, 1024], BF16, name="t_sq", tag="sq")
            rn0 = rnp.tile([1, 1024], BF16, name="t_rn", tag="rn")
            for h in range(2):
                for dh in range(2):
                    head_chain(0, th0, tsq0, h, dh)
                norm_half(tsq0, rn0, h)
                stage_finish(0, th0, rn0, h)

            tx0 = phase(0, 0, 0, nxt=1)
            # diag exp values: exp(10 * |u8_i|^2) == exp(10 * sim_ii)
            colreduce_exp((t_on[0], t_on[0]), dg_st, 10.0)
            tx1 = phase(1, 0, 1, nxt=2)
            tx2 = phase(2, 0, 2, nxt=3, cs_prev=(1, tx1))
            # pos: possim_i = sum_d u0[d,i]*u3[d,i]; host uses 10*possim
            colreduce_exp((t_on[0], t_on[3]), ps_st, None)
            tx3 = phase(3, 1, 3, cs_prev=(2, tx2))
            tx4 = phase(4, 0, 3, cs_prev=(3, tx3))
            block_cs(4, tx4)


            nc.gpsimd.dma_start(o_rp[:],
                                rp_st[:].rearrange("p a b -> p (a b)"))
            nc.gpsimd.dma_start(o_cp.rearrange("a r -> (a r)")[None, :],
                                cp_st[:])
            nc.gpsimd.dma_start(o_dg[:], dg_st[:])
            nc.gpsimd.dma_start(o_ps[:], ps_st[:])

    try:
        nc.compile()
    finally:
        bacc.get_activation_tables = _orig_gat
    _CACHE["nc"] = nc
    return nc


def _host_inputs(embedded_data, W, b):
    emb = np.asarray(embedded_data, dtype=np.float32)      # [8192, 2048]
    W = np.asarray(W, dtype=np.float32)
    b = np.asarray(b, dtype=np.float32)
    # slab s tile layout: [128(p), 16(kc), 1024(r)], value = emb[r0+r, 128*kc+p]
    embT = np.ascontiguousarray(emb.T)                     # [2048, 8192]
    emb8 = embT.reshape(16, 128, 8192).transpose(1, 0, 2)  # [128, 16, 8192]
    emb8 = emb8.astype(ml_dtypes.float8_e4m3)
    W8 = (W * WSCALE).reshape(16, 128, 256).transpose(1, 0, 2)
    W8 = np.ascontiguousarray(W8).astype(ml_dtypes.float8_e4m3)
    bS = np.ascontiguousarray((b * WSCALE).reshape(2, 128).T).astype(
        np.float32)
    ones_col = np.ones((128, 1), ml_dtypes.bfloat16)
    ones_row = np.ones((1, 512), ml_dtypes.bfloat16)
    ones8 = np.ones((128, 32), ml_dtypes.float8_e5m2)
    in_maps = []
    for c in range(8):
        sl = np.stack([emb8[:, :, 1024 * s:1024 * (s + 1)] for s in SLOTS[c]],
                      axis=1)                              # [128, 4, 16, 1024]
        in_maps.append({"emb8": np.ascontiguousarray(sl), "W8": W8, "bS": bS,
                        "ones_col": ones_col, "ones_row": ones_row,
                        "ones8": ones8})
    return in_maps


def _combine(results):
    neg = np.zeros(8192, np.float64)
    pos = np.zeros(8192, np.float64)
    for c in range(8):
        S = SLOTS[c]
        rp = results[c]["rowpart"].astype(np.float64)
        rp = rp.reshape(128, 5, 8).transpose(1, 2, 0).reshape(5, 1024)
        cp = results[c]["colpart"].astype(np.float64)
        dg = results[c]["diagexp"].astype(np.float64).ravel()
        sl = [np.s_[1024 * s:1024 * (s + 1)] for s in S]
        neg[sl[0]] += rp[0] - dg          # diag block, self-sim removed
        neg[sl[0]] += rp[1]; neg[sl[1]] += cp[0]   # B1 (0,1)
        neg[sl[0]] += rp[2]; neg[sl[2]] += cp[1]   # B2 (0,2)
        neg[sl[1]] += rp[3]; neg[sl[3]] += cp[2]   # B3 (1,3)
        if c < 4:                                   # B4 (0,3) dedup: cores 0-3
            neg[sl[0]] += rp[4]; neg[sl[3]] += cp[3]
            ps = results[c]["possim"].astype(np.float64).ravel()
            pos[sl[0]] = ps
            pos[1024 * S[3]:1024 * (S[3] + 1)] = ps
    loss = -np.mean(10.0 * pos - np.log(neg))
    return np.float32(loss)


def run(embedded_data, W, b, trace=False):
    from concourse import bass_utils
    nc = _build()
    in_maps = _host_inputs(embedded_data, W, b)
    res = bass_utils.run_bass_kernel_spmd(nc, in_maps, core_ids=list(range(8)),
                                          trace=trace)
    return _combine(res.results), res


def kernel(embedded_data, W, b):
    loss, _ = run(embedded_data, W, b, trace=False)
    return np.asarray(loss, dtype=np.float32)
